# revision 6
# baseline (speedup 1.0000x reference)
"""Radius-graph adjacency mask (radius_graph r=3, loop=True) on 8 TRN2 NeuronCores.

Strategy
--------
mask[i, j] = (||p_i - p_j||^2 <= R2)  for pos [8192, 3].

val(i, j) = (R2 + eps) - d2(i, j) is computed as a single small-K matmul:
    val = sum_r q_rows[r, i] * k_rows[r, j]
where the q/k rows hold 3-way bf16 splits of the augmented query/key vectors
(2x, sq terms), so the bf16 TensorE matmul (1 cycle/row) reproduces the fp32
value to ~24-bit accuracy.  PSUM then holds val; mask = (val >= 0) via
VectorE is_ge / ScalarE Sign (both engines share the PSUM-read load), written
as int8 and DMA'd out.

Sharding: rows data-parallel across 8 cores (1024 query rows each).  In sorted
mode the atoms are z-sorted so each 128-query block only needs a W-wide window
of keys (all |z_i - z_j| <= 3 neighbors are inside), cutting the per-core slab
from [1024, 8192] to [1024, W].  The host scatters the slabs back into the
full [8192, 8192] bool mask.
"""

from contextlib import ExitStack

import ml_dtypes
import numpy as np

import concourse.bass as bass
import concourse.mybir as mybir
import concourse.tile as tile
from concourse import bacc
from concourse.bass_utils import run_bass_kernel_spmd

N = 8192
R2 = 9.0
RADIUS = 3.0
EPS = 1e-5
NCORES = 8
P = 128
KP = 32                       # padded contraction rows (30 used)
BLOCKS = (N // NCORES) // P   # 8 query blocks of 128 rows per core
BF16 = ml_dtypes.bfloat16

SORT_MODE = True              # z-sorted banded windows (falls back to dense)


def _bf16_split3(x):
    """Split f64 array into 3 bf16 components summing to ~24-bit accuracy."""
    b0 = x.astype(BF16)
    r1 = x - b0.astype(np.float64)
    b1 = r1.astype(BF16)
    r2 = r1 - b1.astype(np.float64)
    b2 = r2.astype(BF16)
    return b0.astype(np.float64), b1.astype(np.float64), b2.astype(np.float64)


def _build_rows(ps):
    """Build the KP-row augmented query/key matrices (f64 holding bf16 values).

    val = sum_r q_rows[r, i] * k_rows[r, j] = (R2 + EPS) - d2(i, j)
    """
    n = ps.shape[0]
    A = 2.0 * ps.T                      # (3, n) query-side coefficient
    B = ps.T                            # (3, n) key-side
    S = (R2 + EPS) - (ps * ps).sum(1)   # query-side constant term
    T = -(ps * ps).sum(1)               # key-side constant term
    ones = np.ones(n)

    rows_q, rows_k = [], []
    for c in range(3):
        Asp = _bf16_split3(A[c])
        Bsp = _bf16_split3(B[c])
        # all split-product terms above ~2^-32 relative (drop (2,2) only)
        for u, v in [(0, 0), (0, 1), (1, 0), (1, 1), (0, 2), (2, 0), (1, 2), (2, 1)]:
            rows_q.append(Asp[u])
            rows_k.append(Bsp[v])
    for s in _bf16_split3(S):
        rows_q.append(s)
        rows_k.append(ones)
    for t in _bf16_split3(T):
        rows_q.append(ones)
        rows_k.append(t)

    q = np.zeros((KP, n))
    k = np.zeros((KP, n))
    q[: len(rows_q)] = np.stack(rows_q)
    k[: len(rows_k)] = np.stack(rows_k)
    return q, k


def _build_graph(W, nslab, slab_of_block):
    """Build the SPMD Bass graph (same for every core).

    Inputs (per core):
      q   [128, BLOCKS, 128] bf16 : partition 32*j + r holds q_rows[r] for the
                                    block's queries, replicated over j=0..3.
      k   [128, nslab, W//4] bf16 : partition 32*j + r holds k_rows[r] for
                                    column quarter j of the slab window.
    Output:
      out [BLOCKS, 128, W] int8   : 1 where mask else 0/-1 (decode == 1).
    """
    WQ = W // 4          # columns per quarter
    PSUM_FD = 2048       # psum tile free dim (4 banks)
    assert W % 2048 == 0 and WQ % 512 == 0

    nc = bacc.Bacc("TRN2", target_bir_lowering=False)
    q_ext = nc.declare_dram_parameter("q", [P, BLOCKS, P], mybir.dt.bfloat16, isOutput=False)
    k_ext = nc.declare_dram_parameter("k", [P, nslab, WQ], mybir.dt.bfloat16, isOutput=False)
    out_ext = nc.declare_dram_parameter("out", [BLOCKS, P, W], mybir.dt.int8, isOutput=True)

    with tile.TileContext(nc) as tc, ExitStack() as ctx:
        kpool = ctx.enter_context(tc.tile_pool(name="keys", bufs=1))
        qpool = ctx.enter_context(tc.tile_pool(name="queries", bufs=1))
        psum = ctx.enter_context(tc.tile_pool(name="psum", bufs=2, space="PSUM"))
        mpool = ctx.enter_context(tc.tile_pool(name="mask", bufs=4))

        q_sb = qpool.tile([P, BLOCKS, P], mybir.dt.bfloat16)
        nc.sync.dma_start(q_sb[:], q_ext[:])
        k_sb = kpool.tile([P, nslab, WQ], mybir.dt.bfloat16)
        for s in range(nslab):
            nc.sync.dma_start(k_sb[:, s], k_ext[:, s])

        tidx = 0
        for b in range(BLOCKS):
            s = slab_of_block[b]
            for h in range(W // PSUM_FD):
                pt = psum.tile([P, PSUM_FD], mybir.dt.float32)
                for t2 in range(PSUM_FD // 512):
                    col0 = h * PSUM_FD + t2 * 512   # column offset within W
                    j = col0 // WQ                  # quarter -> PE row group
                    qcol = col0 % WQ
                    nc.tensor.matmul(
                        pt[:, t2 * 512 : (t2 + 1) * 512],
                        lhsT=q_sb[32 * j : 32 * (j + 1), b, :],
                        rhs=k_sb[32 * j : 32 * (j + 1), s, qcol : qcol + 512],
                        start=True,
                        stop=True,
                        tile_position=(32 * j, 0),
                    )
                mt = mpool.tile([P, PSUM_FD], mybir.dt.int8)
                if tidx % 2 == 0:
                    nc.scalar.activation(mt[:], pt[:], mybir.ActivationFunctionType.Sign)
                else:
                    nc.vector.tensor_scalar(mt[:], pt[:], 0.0, None, mybir.AluOpType.is_ge)
                tidx += 1
                nc.sync.dma_start(out_ext[b, :, h * PSUM_FD : (h + 1) * PSUM_FD], mt[:])
    nc.compile()
    return nc


def _quarters(k32):
    """[32, W] -> [128, W//4] with quarter j at partitions 32j..32j+31."""
    W = k32.shape[1]
    return k32.reshape(KP, 4, W // 4).transpose(1, 0, 2).reshape(P, W // 4)


def _prepare(pos):
    """Host prep: sort, windows, split rows, per-core in_maps."""
    posf = np.asarray(pos, dtype=np.float64)
    nblocks = N // P

    # recenter: d2 is translation-invariant, but smaller |coords| shrink the
    # fp32 cancellation error in sq_i + sq_j - 2 x.y by ~4x
    posf = posf - (posf.min(0) + posf.max(0)) / 2.0

    use_sort = SORT_MODE
    if use_sort:
        order = np.argsort(posf[:, 2], kind="stable")
        ps = posf[order]
        z = ps[:, 2]
        zb = z.reshape(nblocks, P)
        ilo = np.searchsorted(z, zb.min(1) - RADIUS, side="left")
        ihi = np.searchsorted(z, zb.max(1) + RADIUS, side="right")
        wmax = int((ihi - ilo).max())
        W = max(2048, -(-wmax // 2048) * 2048)
        if W >= N:
            use_sort = False
    if not use_sort:
        order = np.arange(N)
        ps = posf
        W = N
        ilo = np.zeros(nblocks, dtype=np.int64)

    off = np.clip(ilo, 0, N - W).astype(np.int64)
    qrows, krows = _build_rows(ps)           # (32, N) f64 over sorted order
    q16 = qrows.astype(BF16)
    k16 = krows.astype(BF16)

    nslab = BLOCKS if use_sort else 1
    in_maps = []
    for c in range(NCORES):
        qc = np.zeros((P, BLOCKS, P), dtype=BF16)
        kc = np.zeros((P, nslab, W // 4), dtype=BF16)
        for b in range(BLOCKS):
            g = c * BLOCKS + b
            qb = q16[:, g * P : (g + 1) * P]          # [32, 128]
            qc[:, b, :] = np.tile(qb, (4, 1))         # replicate to 4 groups
            if use_sort:
                kc[:, b, :] = _quarters(k16[:, off[g] : off[g] + W])
        if not use_sort:
            kc[:, 0, :] = _quarters(k16)
        in_maps.append({"q": qc, "k": kc})

    slab_of_block = list(range(BLOCKS)) if use_sort else [0] * BLOCKS
    return order, off, W, nslab, slab_of_block, in_maps


LAST_RESULTS = None  # BassKernelResults of the most recent run (for profiling)


def kernel(pos):
    global LAST_RESULTS
    order, off, W, nslab, slab_of_block, in_maps = _prepare(pos)
    nc = _build_graph(W, nslab, slab_of_block)
    res = run_bass_kernel_spmd(nc, in_maps, list(range(NCORES)))
    LAST_RESULTS = res

    full = np.zeros((N, N), dtype=bool)
    for c in range(NCORES):
        o = res.results[c]["out"]                      # [BLOCKS, 128, W] int8
        for b in range(BLOCKS):
            g = c * BLOCKS + b
            rows = order[g * P : (g + 1) * P]
            cols = order[off[g] : off[g] + W]
            full[np.ix_(rows, cols)] = o[b] == 1
    return full
